# revision 34
# baseline (speedup 1.0000x reference)
import sys
sys.path.insert(0, "/opt/trn_rl_repo")
import math
import numpy as np
import ml_dtypes

import concourse.bass as bass
from concourse import bacc, mybir
from concourse.tile import TileContext
from concourse.bass_utils import run_bass_kernel_spmd
from concourse.masks import make_identity

F32 = mybir.dt.float32
F32R = mybir.dt.float32r
FP8 = mybir.dt.float8e4
BF16 = mybir.dt.bfloat16
I8 = mybir.dt.int8
AF = mybir.ActivationFunctionType
ALU = mybir.AluOpType
AX = mybir.AxisListType

N, G, E = 16384, 32, 524288
D, DFF, ZI, K, L = 512, 1024, 64, 4, 4
UMAP_A, UMAP_B = 1.577, 0.8951
BN_EPS = 1e-5
NCORES = 8
NL = N // NCORES      # 2048 local nodes per core
GL = G // NCORES      # 4 local graphs per core
NG = N // G           # 512 nodes per graph
KB = N // 128         # 128 source blocks

_NC_CACHE = None


def build_nc():
    nc = bacc.Bacc("TRN2", target_bir_lowering=False, debug=False,
                   enable_asserts=True, num_devices=NCORES)

    xt = nc.dram_tensor("xt", (10, NL), F32R, kind="ExternalInput")
    xn = nc.dram_tensor("xn", (128, KB * 10), BF16, kind="ExternalInput")
    acm = nc.dram_tensor("acm", (N, NL), FP8, kind="ExternalInput")
    embw = nc.dram_tensor("embw", (10, D), F32R, kind="ExternalInput")
    gw1 = nc.dram_tensor("gw1", (L * D, D), F32R, kind="ExternalInput")
    gw2 = nc.dram_tensor("gw2", (L * D, D), F32R, kind="ExternalInput")
    mw1 = nc.dram_tensor("mw1", (D, DFF), F32R, kind="ExternalInput")
    mw2 = nc.dram_tensor("mw2", (DFF, DFF), F32R, kind="ExternalInput")
    mw3 = nc.dram_tensor("mw3", (DFF, ZI), F32R, kind="ExternalInput")
    hw1 = nc.dram_tensor("hw1", (K * ZI, ZI), F32R, kind="ExternalInput")
    hw2 = nc.dram_tensor("hw2", (K * ZI, ZI), F32R, kind="ExternalInput")
    gb1_d = nc.dram_tensor("gb1_d", (128, 16), F32, kind="ExternalInput")
    bng_d = nc.dram_tensor("bng_d", (128, 16), F32, kind="ExternalInput")
    bnb_d = nc.dram_tensor("bnb_d", (128, 16), F32, kind="ExternalInput")
    mb1_d = nc.dram_tensor("mb1_d", (128, 8), F32, kind="ExternalInput")
    mb2_d = nc.dram_tensor("mb2_d", (128, 8), F32, kind="ExternalInput")
    mb3_d = nc.dram_tensor("mb3_d", (ZI, 1), F32, kind="ExternalInput")
    hb1_d = nc.dram_tensor("hb1_d", (ZI, K), F32, kind="ExternalInput")
    hb2_d = nc.dram_tensor("hb2_d", (ZI, K), F32, kind="ExternalInput")
    mb1_r = nc.dram_tensor("mb1_r", (1, DFF), F32R, kind="ExternalInput")
    mb2_r = nc.dram_tensor("mb2_r", (1, DFF), F32R, kind="ExternalInput")
    hb1_r = nc.dram_tensor("hb1_r", (1, K * ZI), F32R, kind="ExternalInput")
    qout = nc.dram_tensor("qout", (GL * K * 4 * 128, NG), F32,
                          kind="ExternalOutput")

    with TileContext(nc) as tc:
        with (
            tc.tile_pool(name="const", bufs=1) as cp,
            tc.tile_pool(name="res", bufs=1) as rp,
            tc.tile_pool(name="ps", bufs=1, space="PSUM") as ps,
            tc.tile_pool(name="dram", bufs=1, space="DRAM") as dp,
        ):
            ident = cp.tile([128, 128], F32, tag="ident")
            make_identity(nc, ident[:])
            nla = cp.tile([128, 1], F32, tag="nla")
            nc.gpsimd.memset(nla[:], -math.log(UMAP_A))
            ofn = cp.tile([64, 1], F32, tag="ofn")
            nc.gpsimd.memset(ofn[:], -0.5)
            onfb = cp.tile([1, NG], BF16, tag="onfb")
            nc.gpsimd.memset(onfb[:], 1.0)
            onf_f = cp.tile([1, NG], F32, tag="onf_f")
            nc.gpsimd.memset(onf_f[:], 1.0)
            onf = cp.tile([1, NG], F32R, tag="onf")
            nc.vector.tensor_copy(onf[:], onf_f[:])

            gb1w = cp.tile([128, 16], F32, tag="gb1w")
            nc.sync.dma_start(gb1w[:], gb1_d[:, :])
            bngw = cp.tile([128, 16], F32, tag="bngw")
            nc.sync.dma_start(bngw[:], bng_d[:, :])
            bnbw = cp.tile([128, 16], F32, tag="bnbw")
            nc.sync.dma_start(bnbw[:], bnb_d[:, :])
            mb1w = cp.tile([128, 8], F32, tag="mb1w")
            nc.sync.dma_start(mb1w[:], mb1_d[:, :])
            mb2w = cp.tile([128, 8], F32, tag="mb2w")
            nc.sync.dma_start(mb2w[:], mb2_d[:, :])
            mb3w = cp.tile([ZI, 1], F32, tag="mb3w")
            nc.sync.dma_start(mb3w[:], mb3_d[:, :])
            hb1w = cp.tile([ZI, K], F32, tag="hb1w")
            nc.sync.dma_start(hb1w[:], hb1_d[:, :])
            hb2w = cp.tile([ZI, K], F32, tag="hb2w")
            nc.sync.dma_start(hb2w[:], hb2_d[:, :])
            mb1rw = cp.tile([1, DFF], F32R, tag="mb1rw")
            nc.sync.dma_start(mb1rw[:], mb1_r[:, :])
            mb2rw = cp.tile([1, DFF], F32R, tag="mb2rw")
            nc.sync.dma_start(mb2rw[:], mb2_r[:, :])
            hb1rw = cp.tile([1, K * ZI], F32R, tag="hb1rw")
            nc.sync.dma_start(hb1rw[:], hb1_r[:, :])

            hT = [rp.tile([128, NL], F32R, tag=f"hT{fc}", name=f"hT{fc}")
                  for fc in range(4)]

            # ping-pong chunked AllGather buffers (bf16, per feature chunk)
            agin = [[dp.tile([NL, 128], BF16, tag=f"agi{s}_{fc}",
                             name=f"agi{s}_{fc}") for fc in range(4)]
                    for s in range(3)]
            agout = [[dp.tile([N, 128], BF16, tag=f"ago{s}_{fc}",
                              name=f"ago{s}_{fc}", addr_space="Shared")
                      for fc in range(4)]
                     for s in range(3)]
            bn_loc = [dp.tile([128, 8], F32, tag=f"bl{i}", name=f"bl{i}")
                      for i in range(L)]
            bn_glob = [dp.tile([128, 8], F32, tag=f"bg{i}", name=f"bg{i}",
                           addr_space="Shared")
                       for i in range(L)]

            # ---------------- GIN layers (incl. embedding + L1 x-trick) ----
            # h0 = x_aug @ embw_aug ; agg0 = (A^T x_aug) @ embw_aug
            # so mt(l=0) = embw_aug^T @ (x_aug^T + (A^T x_aug)^T)
            with tc.tile_pool(name="gin", bufs=1) as gp:
                mt = [gp.tile([128, NL], F32R, tag=f"mt{fc}", name=f"mt{fc}")
                      for fc in range(4)]
                u2T = [gp.tile([128, NL], F32R, tag=f"u2_{fc}",
                               name=f"u2_{fc}") for fc in range(4)]

                xt_sb = gp.tile([10, NL], F32R, tag="xt")
                nc.sync.dma_start(xt_sb[:], xt[:, :])
                ew_sb = gp.tile([10, D], F32R, tag="ew")
                nc.sync.dma_start(ew_sb[:], embw[:, :])
                xn_sb = gp.tile([128, KB * 10], BF16, tag="xn")
                nc.sync.dma_start(xn_sb[:], xn[:, :])

                # hT0 = embw_aug^T @ x_aug^T(local)
                for fc in range(4):
                    for j in range(4):
                        p = ps.tile([128, 512], F32, tag=f"b{4 + fc}")
                        nc.tensor.matmul(p[:],
                                         ew_sb[:, 128 * fc:128 * fc + 128],
                                         xt_sb[:, 512 * j:512 * j + 512],
                                         start=True, stop=True)
                        nc.vector.tensor_copy(hT[fc][:, 512 * j:512 * j + 512],
                                              p[:])

                # xaT = (A^T x_aug)^T  [10, NL]
                pa = [ps.tile([10, 512], F32, tag=f"b{dc}", name=f"pa{dc}")
                      for dc in range(4)]
                for g2 in range(KB // 2):
                    ar0 = gp.tile([128, 2 * NL], FP8, tag="ai", bufs=2)
                    eng = nc.scalar if g2 % 2 == 0 else nc.sync
                    eng.dma_start(
                        ar0[:],
                        acm[256 * g2:256 * g2 + 256, :]
                        .rearrange("(kk p) c -> p kk c", p=128))
                    for kk in range(2):
                        k = 2 * g2 + kk
                        for dc in range(4):
                            nc.tensor.matmul(
                                pa[dc][:],
                                xn_sb[:, 10 * k:10 * k + 10],
                                ar0[:, NL * kk + 512 * dc:
                                    NL * kk + 512 * dc + 512],
                                start=(k == 0), stop=(k == KB - 1))
                s_sb = gp.tile([10, NL], F32R, tag="s")
                for dc in range(4):
                    nc.vector.tensor_tensor(
                        out=s_sb[:, 512 * dc:512 * dc + 512],
                        in0=pa[dc][:],
                        in1=xt_sb[:, 512 * dc:512 * dc + 512],
                        op=ALU.add)

                # mt(l=0) = embw_aug^T @ s
                for fc in range(4):
                    for j in range(4):
                        p = ps.tile([128, 512], F32, tag=f"b{4 + fc}")
                        nc.tensor.matmul(p[:],
                                         ew_sb[:, 128 * fc:128 * fc + 128],
                                         s_sb[:, 512 * j:512 * j + 512],
                                         start=True, stop=True)
                        nc.vector.tensor_copy(mt[fc][:, 512 * j:512 * j + 512],
                                              p[:])

                for l in range(L):
                    w1s = gp.tile([128, 2048], F32R, tag="w1")
                    w2s = gp.tile([128, 2048], F32R, tag="w2")
                    for ic in range(4):
                        r0 = 512 * l + 128 * ic
                        nc.sync.dma_start(w1s[:, 512 * ic:512 * ic + 512],
                                          gw1[r0:r0 + 128, :])
                        nc.sync.dma_start(w2s[:, 512 * ic:512 * ic + 512],
                                          gw2[r0:r0 + 128, :])

                    # aggregation for l>=1: mt = h_table^T @ A + hT
                    # fc-pair passes: pass fp_ covers feature chunks
                    # {2fp_, 2fp_+1} over the FULL dst width, so the first
                    # pass starts as soon as AG chunks 0,1 have landed and
                    # each stationary load feeds 4 consecutive matmuls.
                    if l > 0:
                        pp = l - 1
                        for fp_ in range(2):
                            pb = [ps.tile([128, 512], F32, tag=f"b{i}",
                                          name=f"pb{i}") for i in range(8)]
                            for g4 in range(KB // 4):
                                tch = [gp.tile([128, 512], BF16,
                                               tag=f"tc{fci}", bufs=3,
                                               name=f"tch{fci}")
                                       for fci in range(2)]
                                for fci in range(2):
                                    fc = 2 * fp_ + fci
                                    nc.sync.dma_start(
                                        tch[fci][:],
                                        agout[pp][fc]
                                        [512 * g4:512 * g4 + 512, :]
                                        .rearrange("(kk p) c -> p kk c",
                                                   p=128))
                                ar = gp.tile([128, 4 * NL], FP8, tag="ar4",
                                             bufs=2)
                                nc.scalar.dma_start(
                                    ar[:],
                                    acm[512 * g4:512 * g4 + 512, :]
                                    .rearrange("(kk p) c -> p kk c", p=128))
                                for kk in range(4):
                                    k = 4 * g4 + kk
                                    for fci in range(2):
                                        for dc in range(4):
                                            nc.tensor.matmul(
                                                pb[fci * 4 + dc][:],
                                                tch[fci][:, 128 * kk:
                                                         128 * kk + 128],
                                                ar[:, NL * kk + 512 * dc:
                                                   NL * kk + 512 * dc + 512],
                                                start=(k == 0),
                                                stop=(k == KB - 1))
                            for fci in range(2):
                                fc = 2 * fp_ + fci
                                for dc in range(4):
                                    col = 512 * dc
                                    nc.vector.tensor_tensor(
                                        out=mt[fc][:, col:col + 512],
                                        in0=pb[fci * 4 + dc][:],
                                        in1=hT[fc][:, col:col + 512],
                                        op=ALU.add)

                    # GIN MLP: u1 = relu(m@w1+b1); u2 = u1@w2
                    for j in range(4):
                        ncol = 512 * j
                        u1c = [gp.tile([128, 512], F32R, tag=f"u1_{oc}",
                                       bufs=2, name=f"u1c{oc}")
                               for oc in range(4)]
                        for oc in range(4):
                            p = ps.tile([128, 512], F32, tag=f"b{oc}")
                            for ic in range(4):
                                nc.tensor.matmul(
                                    p[:],
                                    w1s[:, 512 * ic + 128 * oc:
                                        512 * ic + 128 * oc + 128],
                                    mt[ic][:, ncol:ncol + 512],
                                    start=(ic == 0), stop=(ic == 3))
                            nc.scalar.activation(
                                u1c[oc][:], p[:], AF.Relu,
                                bias=gb1w[:, 4 * l + oc:4 * l + oc + 1])
                        for oc in range(4):
                            p = ps.tile([128, 512], F32, tag=f"b{4 + oc}")
                            for ic in range(4):
                                nc.tensor.matmul(
                                    p[:],
                                    w2s[:, 512 * ic + 128 * oc:
                                        512 * ic + 128 * oc + 128],
                                    u1c[ic][:],
                                    start=(ic == 0), stop=(ic == 3))
                            nc.vector.tensor_copy(u2T[oc][:, ncol:ncol + 512],
                                                  p[:])

                    # BN stats (local sums) -> AllReduce
                    stat = gp.tile([128, 8], F32, tag="stat")
                    for fc in range(4):
                        nc.vector.reduce_sum(stat[:, fc:fc + 1], u2T[fc][:],
                                             axis=AX.X)
                        qacc = gp.tile([128, 1], F32, tag="qacc")
                        for j in range(4):
                            sq = gp.tile([128, 512], F32, tag="sq", bufs=2)
                            nc.scalar.activation(
                                sq[:], u2T[fc][:, 512 * j:512 * j + 512],
                                AF.Square)
                            qp = gp.tile([128, 1], F32, tag=f"qp{j}")
                            nc.vector.reduce_sum(qp[:], sq[:], axis=AX.X)
                            if j == 0:
                                nc.vector.tensor_copy(qacc[:], qp[:])
                            else:
                                nc.vector.tensor_tensor(
                                    out=qacc[:], in0=qp[:], in1=qacc[:],
                                    op=ALU.add)
                        nc.vector.tensor_copy(stat[:, 4 + fc:5 + fc], qacc[:])
                    nc.sync.dma_start(bn_loc[l][:, :], stat[:])
                    nc.gpsimd.collective_compute(
                        "AllReduce", ALU.add,
                        ins=[bn_loc[l][:, :].opt()],
                        outs=[bn_glob[l][:, :].opt()],
                        replica_groups=[list(range(NCORES))],
                    )
                    # keep-warm matmuls: reading `statr` delays them to
                    # AR-trigger time so the PE stays busy (and HAM stays at
                    # full clock) through the AllReduce wait. Results unused.
                    statr = gp.tile([128, 8], F32R, tag="statr")
                    nc.vector.tensor_copy(statr[:], stat[:])
                    for jm in range(32):
                        pj = ps.tile([8, 512], F32, tag=f"b{jm % 2}",
                                     name="junk")
                        nc.tensor.matmul(pj[:], statr[:],
                                         w1s[:, 0:512],
                                         start=True, stop=True)
                    ga = gp.tile([128, 8], F32, tag="ga")
                    nc.sync.dma_start(ga[:], bn_glob[l][:, :])

                    # BN apply + relu + residual into hT, then chunked
                    # AllGather of the new h (bf16), one call per 128-feat
                    # chunk so the next layer's aggregation can start on
                    # chunk 0 while chunks 1-3 are still in flight.
                    for fc in range(4):
                        mu = gp.tile([128, 1], F32, tag="mu")
                        nc.vector.tensor_scalar(out=mu[:], in0=ga[:, fc:fc + 1],
                                                scalar1=1.0 / N, scalar2=None,
                                                op0=ALU.mult)
                        ex2 = gp.tile([128, 1], F32, tag="ex2")
                        nc.vector.tensor_scalar(out=ex2[:],
                                                in0=ga[:, 4 + fc:5 + fc],
                                                scalar1=1.0 / N, scalar2=None,
                                                op0=ALU.mult)
                        mu2 = gp.tile([128, 1], F32, tag="mu2")
                        nc.vector.tensor_tensor(out=mu2[:], in0=mu[:],
                                                in1=mu[:], op=ALU.mult)
                        var = gp.tile([128, 1], F32, tag="var")
                        nc.vector.tensor_tensor(out=var[:], in0=ex2[:],
                                                in1=mu2[:], op=ALU.subtract)
                        vare = gp.tile([128, 1], F32, tag="vare")
                        nc.vector.tensor_scalar(out=vare[:], in0=var[:],
                                                scalar1=BN_EPS, scalar2=None,
                                                op0=ALU.add)
                        std = gp.tile([128, 1], F32, tag="std")
                        nc.scalar.activation(std[:], vare[:], AF.Sqrt)
                        inv = gp.tile([128, 1], F32, tag="inv")
                        nc.vector.reciprocal(inv[:], std[:])
                        sv = gp.tile([128, 1], F32, tag="sv")
                        nc.vector.tensor_tensor(
                            out=sv[:], in0=inv[:],
                            in1=bngw[:, 4 * l + fc:4 * l + fc + 1],
                            op=ALU.mult)
                        mst = gp.tile([128, 1], F32, tag="mst")
                        nc.vector.tensor_tensor(out=mst[:], in0=mu[:],
                                                in1=sv[:], op=ALU.mult)
                        tv = gp.tile([128, 1], F32, tag="tv")
                        nc.vector.tensor_tensor(
                            out=tv[:], in0=bnbw[:, 4 * l + fc:4 * l + fc + 1],
                            in1=mst[:], op=ALU.subtract)
                        for j in range(4):
                            ncol = 512 * j
                            rt = gp.tile([128, 512], F32R, tag="rt", bufs=2)
                            nc.scalar.activation(
                                rt[:], u2T[fc][:, ncol:ncol + 512], AF.Relu,
                                bias=tv[:, 0:1], scale=sv[:, 0:1])
                            nc.vector.tensor_tensor(
                                out=hT[fc][:, ncol:ncol + 512], in0=rt[:],
                                in1=hT[fc][:, ncol:ncol + 512], op=ALU.add)

                        if l < L - 1:
                            pp2 = l
                            for nb4 in range(4):
                                hn = gp.tile([128, 512], BF16, tag="hn",
                                             bufs=2)
                                for nn in range(4):
                                    nb = 4 * nb4 + nn
                                    pt = ps.tile([128, 128], F32,
                                                 tag=f"b{nn % 4}")
                                    nc.tensor.transpose(
                                        pt[:],
                                        hT[fc][:, 128 * nb:128 * nb + 128]
                                        .bitcast(F32),
                                        ident[:])
                                    nc.vector.tensor_copy(
                                        hn[:, 128 * nn:128 * nn + 128], pt[:])
                                nc.sync.dma_start(
                                    agin[pp2][fc]
                                    [512 * nb4:512 * nb4 + 512, :]
                                    .rearrange("(nn p) c -> p nn c", p=128),
                                    hn[:])
                            nc.gpsimd.collective_compute(
                                "AllGather", ALU.bypass,
                                ins=[agin[pp2][fc][:, :].opt()],
                                outs=[agout[pp2][fc][:, :].opt()],
                                replica_groups=[list(range(NCORES))],
                            )

            # ---------------- final MLP + heads + pairwise ----------------
            with tc.tile_pool(name="fin", bufs=1) as fp:
                z3g = [fp.tile([ZI, 512], F32R, tag=f"z3_{g}", name=f"z3_{g}")
                       for g in range(GL)]
                hw1s = [fp.tile([ZI, ZI], F32R, tag=f"hw1_{k}", name=f"hw1s{k}")
                        for k in range(K)]
                hw2s = [fp.tile([ZI, ZI], F32R, tag=f"hw2_{k}", name=f"hw2s{k}")
                        for k in range(K)]
                for k in range(K):
                    nc.sync.dma_start(hw1s[k][:], hw1[ZI * k:ZI * k + ZI, :])
                    nc.sync.dma_start(hw2s[k][:], hw2[ZI * k:ZI * k + ZI, :])

                # phase A: node MLP down to z3, graph-pairs so each
                # stationary weight load feeds 2 matmuls (graphs innermost).
                with tc.tile_pool(name="finA", bufs=1) as fa:
                    mwa_all = fa.tile([128, 4 * DFF], F32R, tag="mwa")
                    nc.sync.dma_start(
                        mwa_all[:],
                        mw1[:, :].rearrange("(ic p) c -> p ic c", p=128))
                    mwb_all = fa.tile([128, 8 * DFF], F32R, tag="mwb")
                    nc.scalar.dma_start(
                        mwb_all[:],
                        mw2[:, :].rearrange("(ic p) c -> p ic c", p=128))
                    mwc_all = fa.tile([128, 8 * ZI], F32R, tag="mwc")
                    nc.sync.dma_start(
                        mwc_all[:],
                        mw3[:, :].rearrange("(ic p) c -> p ic c", p=128))
                    mwa = [mwa_all[:, DFF * ic:DFF * ic + DFF]
                           for ic in range(4)]
                    mwb = [mwb_all[:, DFF * ic:DFF * ic + DFF]
                           for ic in range(8)]
                    mwc = [mwc_all[:, ZI * ic:ZI * ic + ZI]
                           for ic in range(8)]

                    for gp2 in range(GL // 2):
                        gs = (2 * gp2, 2 * gp2 + 1)
                        z1 = [[fa.tile([128, 512], F32R, tag=f"z1_{gj}_{oc}",
                                       name=f"z1_{gj}_{oc}")
                               for oc in range(8)] for gj in range(2)]
                        for oh in range(2):
                            pb2 = [[ps.tile([128, 512], F32,
                                            tag=f"b{2 * oi + gj}",
                                            name=f"pb2_{oi}_{gj}")
                                    for gj in range(2)] for oi in range(4)]
                            for ic in range(4):
                                for oi in range(4):
                                    oc = 4 * oh + oi
                                    for gj in range(2):
                                        nc.tensor.matmul(
                                            pb2[oi][gj][:],
                                            mwa_all[:, DFF * ic + 128 * oc:DFF * ic + 128 * oc + 128],
                                            hT[ic][:, 512 * gs[gj]:
                                                    512 * gs[gj] + 512],
                                            start=(ic == 0), stop=False)
                            for oi in range(4):
                                oc = 4 * oh + oi
                                for gj in range(2):
                                    nc.tensor.matmul(
                                        pb2[oi][gj][:],
                                        mb1rw[:, 128 * oc:128 * oc + 128],
                                        onf[:, 0:512], start=False, stop=True,
                                        skip_group_check=True)
                                    nc.vector.tensor_scalar(
                                        out=z1[gj][oc][:], in0=pb2[oi][gj][:],
                                        scalar1=0.0, scalar2=None, op0=ALU.max)
                        z2 = [[fa.tile([128, 512], F32R, tag=f"z2_{gj}_{oc}",
                                       name=f"z2_{gj}_{oc}")
                               for oc in range(8)] for gj in range(2)]
                        for oh in range(2):
                            pb2 = [[ps.tile([128, 512], F32,
                                            tag=f"b{2 * oi + gj}",
                                            name=f"pc2_{oi}_{gj}")
                                    for gj in range(2)] for oi in range(4)]
                            for ic in range(8):
                                for oi in range(4):
                                    oc = 4 * oh + oi
                                    for gj in range(2):
                                        nc.tensor.matmul(
                                            pb2[oi][gj][:],
                                            mwb_all[:, DFF * ic + 128 * oc:DFF * ic + 128 * oc + 128],
                                            z1[gj][ic][:],
                                            start=(ic == 0), stop=False)
                            for oi in range(4):
                                oc = 4 * oh + oi
                                for gj in range(2):
                                    nc.tensor.matmul(
                                        pb2[oi][gj][:],
                                        mb2rw[:, 128 * oc:128 * oc + 128],
                                        onf[:, 0:512], start=False, stop=True,
                                        skip_group_check=True)
                                    nc.vector.tensor_scalar(
                                        out=z2[gj][oc][:], in0=pb2[oi][gj][:],
                                        scalar1=0.0, scalar2=None, op0=ALU.max)
                        pz2 = [ps.tile([ZI, 512], F32, tag=f"b{gj}",
                                       name=f"pz2_{gj}")
                               for gj in range(2)]
                        for ic in range(8):
                            for gj in range(2):
                                nc.tensor.matmul(pz2[gj][:],
                                                 mwc[ic],
                                                 z2[gj][ic][:],
                                                 start=(ic == 0),
                                                 stop=(ic == 7))
                        for gj in range(2):
                            nc.vector.tensor_tensor(
                                out=z3g[gs[gj]][:], in0=pz2[gj][:],
                                in1=mb3w[:, 0:1].to_broadcast([ZI, 512])[:],
                                op=ALU.add)

                # phase B/C over two k-halves: heads batched so each
                # stationary load feeds all 4 graphs, then pairwise with
                # batched Ln/Sigmoid (few activation-table swaps).
                with tc.tile_pool(name="finB", bufs=1) as fb:
                    for hf in range(2):
                        idx = [(g, k) for k in (2 * hf, 2 * hf + 1)
                               for g in range(GL)]
                        hkh8 = [fb.tile([ZI, 512], BF16, tag=f"hh{j}",
                                        name=f"hh{j}") for j in range(8)]
                        hkl8 = [fb.tile([ZI, 512], BF16, tag=f"hl{j}",
                                        name=f"hl{j}") for j in range(8)]
                        rsh8 = [fb.tile([1, 512], BF16, tag=f"rh{j}",
                                        name=f"rh{j}") for j in range(8)]
                        rsl8 = [fb.tile([1, 512], BF16, tag=f"rl{j}",
                                        name=f"rl{j}") for j in range(8)]
                        h18 = [fb.tile([ZI, 512], F32R, tag=f"h1_{j}",
                                       name=f"h1_{j}") for j in range(8)]
                        for kj in range(2):
                            k = 2 * hf + kj
                            p14 = [ps.tile([ZI, 512], F32,
                                           tag=f"b{4 * kj + g}",
                                           name=f"p14_{g}")
                                   for g in range(GL)]
                            for g in range(GL):
                                nc.tensor.matmul(p14[g][:], hw1s[k][:],
                                                 z3g[g][:],
                                                 start=True, stop=False)
                            for g in range(GL):
                                nc.tensor.matmul(
                                    p14[g][:], hb1rw[:, ZI * k:ZI * k + ZI],
                                    onf[:, 0:512], start=False, stop=True,
                                    skip_group_check=True)
                                nc.vector.tensor_scalar(
                                    out=h18[4 * kj + g][:], in0=p14[g][:],
                                    scalar1=0.0, scalar2=None, op0=ALU.max)
                        for kj in range(2):
                            k = 2 * hf + kj
                            p24 = [ps.tile([ZI, 512], F32,
                                           tag=f"b{4 * kj + g}",
                                           name=f"p24_{g}")
                                   for g in range(GL)]
                            for g in range(GL):
                                j = 4 * kj + g
                                nc.tensor.matmul(p24[g][:], hw2s[k][:],
                                                 h18[j][:],
                                                 start=True, stop=True)
                            for g in range(GL):
                                j = 4 * kj + g
                                hkt = fb.tile([ZI, 512], F32, tag="hkt",
                                              bufs=2)
                                nc.vector.tensor_tensor(
                                    out=hkt[:], in0=p24[g][:],
                                    in1=hb2w[:, k:k + 1]
                                    .to_broadcast([ZI, 512])[:],
                                    op=ALU.add)
                                nc.vector.tensor_copy(hkh8[j][:], hkt[:])
                                nc.vector.tensor_tensor(out=hkl8[j][:],
                                                        in0=hkt[:],
                                                        in1=hkh8[j][:],
                                                        op=ALU.subtract)
                                sqt = fb.tile([ZI, 512], F32, tag="sqt",
                                              bufs=2)
                                nc.vector.tensor_tensor(out=sqt[:],
                                                        in0=hkt[:],
                                                        in1=hkt[:],
                                                        op=ALU.mult)
                                pr = ps.tile([1, 512], F32, tag=f"b{j % 2}",
                                             name="pr")
                                nc.tensor.matmul(pr[:], ofn[:], sqt[:],
                                                 start=True, stop=True)
                                rsb = fb.tile([1, 512], F32, tag="rsb",
                                              bufs=2)
                                nc.vector.tensor_copy(rsb[:], pr[:])
                                nc.vector.tensor_copy(rsh8[j][:], rsb[:])
                                nc.vector.tensor_tensor(out=rsl8[j][:],
                                                        in0=rsb[:],
                                                        in1=rsh8[j][:],
                                                        op=ALU.subtract)
                        # pairwise: pd = h_i.h_j - r_i/2 - r_j/2 = -d2/2
                        for q in range(4):
                            d2p = [fb.tile([128, 2048], F32, tag=f"d2_{j}",
                                           name=f"d2_{j}") for j in range(2)]
                            for j in range(2):
                                w = 2 * q + j
                                hh_ = hkh8[w]
                                hl_ = hkl8[w]
                                rh_ = rsh8[w]
                                rl_ = rsl8[w]
                                for mb in range(4):
                                    c0 = 128 * mb
                                    pd = ps.tile([128, 512], F32,
                                                 tag=f"b{4 + (j * 4 + mb) % 2}")
                                    nc.tensor.matmul(
                                        pd[:], hh_[:, c0:c0 + 128], hh_[:],
                                        start=True, stop=False)
                                    nc.tensor.matmul(
                                        pd[:], hh_[:, c0:c0 + 128], hl_[:],
                                        start=False, stop=False,
                                        skip_group_check=True)
                                    nc.tensor.matmul(
                                        pd[:], hl_[:, c0:c0 + 128], hh_[:],
                                        start=False, stop=False,
                                        skip_group_check=True)
                                    nc.tensor.matmul(
                                        pd[:], onfb[:, 0:128], rh_[:],
                                        start=False, stop=False,
                                        skip_group_check=True)
                                    nc.tensor.matmul(
                                        pd[:], onfb[:, 0:128], rl_[:],
                                        start=False, stop=False,
                                        skip_group_check=True)
                                    nc.tensor.matmul(
                                        pd[:], rh_[:, c0:c0 + 128], onfb[:],
                                        start=False, stop=False,
                                        skip_group_check=True)
                                    nc.tensor.matmul(
                                        pd[:], rl_[:, c0:c0 + 128], onfb[:],
                                        start=False, stop=True,
                                        skip_group_check=True)
                                    nc.vector.tensor_scalar(
                                        out=d2p[j][:, 512 * mb:512 * mb + 512],
                                        in0=pd[:], scalar1=-2.0,
                                        scalar2=1e-12, op0=ALU.mult,
                                        op1=ALU.max)
                            lnp = [fb.tile([128, 2048], F32, tag=f"ln_{j}",
                                           name=f"ln_{j}") for j in range(2)]
                            for j in range(2):
                                nc.scalar.activation(lnp[j][:], d2p[j][:],
                                                     AF.Ln)
                            for j in range(2):
                                w = 2 * q + j
                                g, k = idx[w]
                                qt = fb.tile([128, 2048], F32, tag="qt",
                                             bufs=2)
                                nc.scalar.activation(qt[:], lnp[j][:],
                                                     AF.Sigmoid,
                                                     bias=nla[:, 0:1],
                                                     scale=-UMAP_B)
                                row = (g * K + k) * 4 * 128
                                nc.sync.dma_start(
                                    qout[row:row + 512, :]
                                    .rearrange("(mb p) c -> p mb c", p=128),
                                    qt[:])
    nc.compile()
    return nc


def _host_prep(inputs):
    x = np.asarray(inputs["x"], np.float32)
    edge_index = np.asarray(inputs["edge_index"], np.int64)
    src, dst = edge_index[0], edge_index[1]

    x_aug = np.concatenate([x, np.ones((N, 1), np.float32)], axis=1)  # [N,10]
    xn_host = np.ascontiguousarray(
        x_aug.reshape(KB, 128, 10).transpose(1, 0, 2).reshape(128, KB * 10)
    ).astype(ml_dtypes.bfloat16)

    shared = {
        "xn": xn_host,
        "embw": np.ascontiguousarray(np.vstack(
            [np.asarray(inputs["emb_w"], np.float32),
             np.asarray(inputs["emb_b"], np.float32)[None, :]])),
        "gw1": np.ascontiguousarray(
            np.asarray(inputs["gin_w1"], np.float32).reshape(L * D, D)),
        "gw2": np.ascontiguousarray(
            np.asarray(inputs["gin_w2"], np.float32).reshape(L * D, D)),
        "mw1": np.ascontiguousarray(np.asarray(inputs["mlp_w1"], np.float32)),
        "mw2": np.ascontiguousarray(np.asarray(inputs["mlp_w2"], np.float32)),
        "mw3": np.ascontiguousarray(np.asarray(inputs["mlp_w3"], np.float32)),
        "hw1": np.ascontiguousarray(
            np.asarray(inputs["head_w1"], np.float32).reshape(K * ZI, ZI)),
        "hw2": np.ascontiguousarray(
            np.asarray(inputs["head_w2"], np.float32).reshape(K * ZI, ZI)),
        "gb1_d": np.ascontiguousarray(
            np.asarray(inputs["gin_b1"], np.float32)
            .reshape(L, 4, 128).transpose(2, 0, 1).reshape(128, 16)),
        "bng_d": np.ascontiguousarray(
            np.asarray(inputs["bn_g"], np.float32)
            .reshape(L, 4, 128).transpose(2, 0, 1).reshape(128, 16)),
        "bnb_d": np.ascontiguousarray(
            np.asarray(inputs["bn_b"], np.float32)
            .reshape(L, 4, 128).transpose(2, 0, 1).reshape(128, 16)),
        "mb1_d": np.ascontiguousarray(
            np.asarray(inputs["mlp_b1"], np.float32).reshape(8, 128).T),
        "mb2_d": np.ascontiguousarray(
            np.asarray(inputs["mlp_b2"], np.float32).reshape(8, 128).T),
        "mb3_d": np.ascontiguousarray(
            np.asarray(inputs["mlp_b3"], np.float32)[:, None]),
        "hb1_d": np.ascontiguousarray(
            np.asarray(inputs["head_b1"], np.float32).T),
        "hb2_d": np.ascontiguousarray(
            np.asarray(inputs["head_b2"], np.float32).T),
        "mb1_r": np.ascontiguousarray(
            np.asarray(inputs["mlp_b1"], np.float32)[None, :]),
        "mb2_r": np.ascontiguousarray(
            np.asarray(inputs["mlp_b2"], np.float32)[None, :]),
        "hb1_r": np.ascontiguousarray(
            np.asarray(inputs["head_b1"], np.float32).reshape(1, K * ZI)),
    }

    in_maps = []
    ones_row = np.ones((1, NL), np.float32)
    for c in range(NCORES):
        lo = NL * c
        mask = (dst >= lo) & (dst < lo + NL)
        flat = src[mask] * NL + (dst[mask] - lo)
        a = np.bincount(flat, minlength=N * NL).astype(np.float32)
        m = dict(shared)
        m["acm"] = np.ascontiguousarray(a.reshape(N, NL)).astype(
            ml_dtypes.float8_e4m3)
        m["xt"] = np.ascontiguousarray(
            np.vstack([x[lo:lo + NL].T, ones_row]))
        in_maps.append(m)
    return in_maps


def kernel(**inputs) -> np.ndarray:
    global _NC_CACHE
    if _NC_CACHE is None:
        _NC_CACHE = build_nc()
    nc = _NC_CACHE
    in_maps = _host_prep(inputs)
    res = run_bass_kernel_spmd(nc, in_maps, core_ids=list(range(NCORES)))
    out = np.concatenate(
        [np.asarray(res.results[c]["qout"]).reshape(GL, K, NG, NG)
         for c in range(NCORES)], axis=0)
    return out


# revision 35
# speedup vs baseline: 1.0039x; 1.0039x over previous
import sys
sys.path.insert(0, "/opt/trn_rl_repo")
import math
import numpy as np
import ml_dtypes

import concourse.bass as bass
from concourse import bacc, mybir
from concourse.tile import TileContext
from concourse.bass_utils import run_bass_kernel_spmd
from concourse.masks import make_identity

F32 = mybir.dt.float32
F32R = mybir.dt.float32r
FP8 = mybir.dt.float8e4
BF16 = mybir.dt.bfloat16
I8 = mybir.dt.int8
AF = mybir.ActivationFunctionType
ALU = mybir.AluOpType
AX = mybir.AxisListType

N, G, E = 16384, 32, 524288
D, DFF, ZI, K, L = 512, 1024, 64, 4, 4
UMAP_A, UMAP_B = 1.577, 0.8951
BN_EPS = 1e-5
NCORES = 8
NL = N // NCORES      # 2048 local nodes per core
GL = G // NCORES      # 4 local graphs per core
NG = N // G           # 512 nodes per graph
KB = N // 128         # 128 source blocks

_NC_CACHE = None


def build_nc():
    nc = bacc.Bacc("TRN2", target_bir_lowering=False, debug=False,
                   enable_asserts=True, num_devices=NCORES)

    xt = nc.dram_tensor("xt", (10, NL), F32R, kind="ExternalInput")
    xn = nc.dram_tensor("xn", (128, KB * 10), BF16, kind="ExternalInput")
    acm = nc.dram_tensor("acm", (N, NL), FP8, kind="ExternalInput")
    embw = nc.dram_tensor("embw", (10, D), F32R, kind="ExternalInput")
    gw1 = nc.dram_tensor("gw1", (L * D, D), F32R, kind="ExternalInput")
    gw2 = nc.dram_tensor("gw2", (L * D, D), F32R, kind="ExternalInput")
    mw1 = nc.dram_tensor("mw1", (D, DFF), F32R, kind="ExternalInput")
    mw2 = nc.dram_tensor("mw2", (DFF, DFF), F32R, kind="ExternalInput")
    mw3 = nc.dram_tensor("mw3", (DFF, ZI), F32R, kind="ExternalInput")
    hw1 = nc.dram_tensor("hw1", (K * ZI, ZI), F32R, kind="ExternalInput")
    hw2 = nc.dram_tensor("hw2", (K * ZI, ZI), F32R, kind="ExternalInput")
    gb1_d = nc.dram_tensor("gb1_d", (128, 16), F32, kind="ExternalInput")
    bng_d = nc.dram_tensor("bng_d", (128, 16), F32, kind="ExternalInput")
    bnb_d = nc.dram_tensor("bnb_d", (128, 16), F32, kind="ExternalInput")
    mb1_d = nc.dram_tensor("mb1_d", (128, 8), F32, kind="ExternalInput")
    mb2_d = nc.dram_tensor("mb2_d", (128, 8), F32, kind="ExternalInput")
    mb3_d = nc.dram_tensor("mb3_d", (ZI, 1), F32, kind="ExternalInput")
    hb1_d = nc.dram_tensor("hb1_d", (ZI, K), F32, kind="ExternalInput")
    hb2_d = nc.dram_tensor("hb2_d", (ZI, K), F32, kind="ExternalInput")
    mb1_r = nc.dram_tensor("mb1_r", (1, DFF), F32R, kind="ExternalInput")
    mb2_r = nc.dram_tensor("mb2_r", (1, DFF), F32R, kind="ExternalInput")
    hb1_r = nc.dram_tensor("hb1_r", (1, K * ZI), F32R, kind="ExternalInput")
    qout = nc.dram_tensor("qout", (GL * K * 4 * 128, NG), F32,
                          kind="ExternalOutput")

    with TileContext(nc) as tc:
        with (
            tc.tile_pool(name="const", bufs=1) as cp,
            tc.tile_pool(name="res", bufs=1) as rp,
            tc.tile_pool(name="ps", bufs=1, space="PSUM") as ps,
            tc.tile_pool(name="dram", bufs=1, space="DRAM") as dp,
        ):
            ident = cp.tile([128, 128], F32, tag="ident")
            make_identity(nc, ident[:])
            nla = cp.tile([128, 1], F32, tag="nla")
            nc.gpsimd.memset(nla[:], -math.log(UMAP_A))
            ofn = cp.tile([64, 1], F32, tag="ofn")
            nc.gpsimd.memset(ofn[:], -0.5)
            onfb = cp.tile([1, NG], BF16, tag="onfb")
            nc.gpsimd.memset(onfb[:], 1.0)
            onf_f = cp.tile([1, NG], F32, tag="onf_f")
            nc.gpsimd.memset(onf_f[:], 1.0)
            onf = cp.tile([1, NG], F32R, tag="onf")
            nc.vector.tensor_copy(onf[:], onf_f[:])

            gb1w = cp.tile([128, 16], F32, tag="gb1w")
            nc.sync.dma_start(gb1w[:], gb1_d[:, :])
            bngw = cp.tile([128, 16], F32, tag="bngw")
            nc.sync.dma_start(bngw[:], bng_d[:, :])
            bnbw = cp.tile([128, 16], F32, tag="bnbw")
            nc.sync.dma_start(bnbw[:], bnb_d[:, :])
            mb1w = cp.tile([128, 8], F32, tag="mb1w")
            nc.sync.dma_start(mb1w[:], mb1_d[:, :])
            mb2w = cp.tile([128, 8], F32, tag="mb2w")
            nc.sync.dma_start(mb2w[:], mb2_d[:, :])
            mb3w = cp.tile([ZI, 1], F32, tag="mb3w")
            nc.sync.dma_start(mb3w[:], mb3_d[:, :])
            hb1w = cp.tile([ZI, K], F32, tag="hb1w")
            nc.sync.dma_start(hb1w[:], hb1_d[:, :])
            hb2w = cp.tile([ZI, K], F32, tag="hb2w")
            nc.sync.dma_start(hb2w[:], hb2_d[:, :])
            mb1rw = cp.tile([1, DFF], F32R, tag="mb1rw")
            nc.sync.dma_start(mb1rw[:], mb1_r[:, :])
            mb2rw = cp.tile([1, DFF], F32R, tag="mb2rw")
            nc.sync.dma_start(mb2rw[:], mb2_r[:, :])
            hb1rw = cp.tile([1, K * ZI], F32R, tag="hb1rw")
            nc.sync.dma_start(hb1rw[:], hb1_r[:, :])

            hT = [rp.tile([128, NL], F32R, tag=f"hT{fc}", name=f"hT{fc}")
                  for fc in range(4)]

            # ping-pong chunked AllGather buffers (bf16, per feature chunk)
            agin = [[dp.tile([NL, 128], BF16, tag=f"agi{s}_{fc}",
                             name=f"agi{s}_{fc}") for fc in range(4)]
                    for s in range(3)]
            agout = [[dp.tile([N, 128], BF16, tag=f"ago{s}_{fc}",
                              name=f"ago{s}_{fc}", addr_space="Shared")
                      for fc in range(4)]
                     for s in range(3)]
            bn_loc = [dp.tile([128, 8], F32, tag=f"bl{i}", name=f"bl{i}")
                      for i in range(L)]
            bn_glob = [dp.tile([128, 8], F32, tag=f"bg{i}", name=f"bg{i}",
                           addr_space="Shared")
                       for i in range(L)]

            # ---------------- GIN layers (incl. embedding + L1 x-trick) ----
            # h0 = x_aug @ embw_aug ; agg0 = (A^T x_aug) @ embw_aug
            # so mt(l=0) = embw_aug^T @ (x_aug^T + (A^T x_aug)^T)
            with tc.tile_pool(name="gin", bufs=1) as gp:
                mt = [gp.tile([128, NL], F32R, tag=f"mt{fc}", name=f"mt{fc}")
                      for fc in range(4)]
                u2T = [gp.tile([128, NL], F32R, tag=f"u2_{fc}",
                               name=f"u2_{fc}") for fc in range(4)]

                xt_sb = gp.tile([10, NL], F32R, tag="xt")
                nc.sync.dma_start(xt_sb[:], xt[:, :])
                ew_sb = gp.tile([10, D], F32R, tag="ew")
                nc.sync.dma_start(ew_sb[:], embw[:, :])
                xn_sb = gp.tile([128, KB * 10], BF16, tag="xn")
                nc.sync.dma_start(xn_sb[:], xn[:, :])

                # hT0 = embw_aug^T @ x_aug^T(local)
                for fc in range(4):
                    for j in range(4):
                        p = ps.tile([128, 512], F32, tag=f"b{4 + fc}")
                        nc.tensor.matmul(p[:],
                                         ew_sb[:, 128 * fc:128 * fc + 128],
                                         xt_sb[:, 512 * j:512 * j + 512],
                                         start=True, stop=True)
                        nc.vector.tensor_copy(hT[fc][:, 512 * j:512 * j + 512],
                                              p[:])

                # xaT = (A^T x_aug)^T  [10, NL]
                pa = [ps.tile([10, 512], F32, tag=f"b{dc}", name=f"pa{dc}")
                      for dc in range(4)]
                for g2 in range(KB // 2):
                    ar0 = gp.tile([128, 2 * NL], FP8, tag="ai", bufs=2)
                    eng = nc.scalar if g2 % 2 == 0 else nc.sync
                    eng.dma_start(
                        ar0[:],
                        acm[256 * g2:256 * g2 + 256, :]
                        .rearrange("(kk p) c -> p kk c", p=128))
                    for kk in range(2):
                        k = 2 * g2 + kk
                        for dc in range(4):
                            nc.tensor.matmul(
                                pa[dc][:],
                                xn_sb[:, 10 * k:10 * k + 10],
                                ar0[:, NL * kk + 512 * dc:
                                    NL * kk + 512 * dc + 512],
                                start=(k == 0), stop=(k == KB - 1))
                s_sb = gp.tile([10, NL], F32R, tag="s")
                for dc in range(4):
                    nc.vector.tensor_tensor(
                        out=s_sb[:, 512 * dc:512 * dc + 512],
                        in0=pa[dc][:],
                        in1=xt_sb[:, 512 * dc:512 * dc + 512],
                        op=ALU.add)

                # mt(l=0) = embw_aug^T @ s
                for fc in range(4):
                    for j in range(4):
                        p = ps.tile([128, 512], F32, tag=f"b{4 + fc}")
                        nc.tensor.matmul(p[:],
                                         ew_sb[:, 128 * fc:128 * fc + 128],
                                         s_sb[:, 512 * j:512 * j + 512],
                                         start=True, stop=True)
                        nc.vector.tensor_copy(mt[fc][:, 512 * j:512 * j + 512],
                                              p[:])

                for l in range(L):
                    w1s = gp.tile([128, 2048], F32R, tag="w1")
                    w2s = gp.tile([128, 2048], F32R, tag="w2")
                    for ic in range(4):
                        r0 = 512 * l + 128 * ic
                        nc.sync.dma_start(w1s[:, 512 * ic:512 * ic + 512],
                                          gw1[r0:r0 + 128, :])
                        nc.sync.dma_start(w2s[:, 512 * ic:512 * ic + 512],
                                          gw2[r0:r0 + 128, :])

                    # aggregation for l>=1: mt = h_table^T @ A + hT
                    # fc-pair passes: pass fp_ covers feature chunks
                    # {2fp_, 2fp_+1} over the FULL dst width, so the first
                    # pass starts as soon as AG chunks 0,1 have landed and
                    # each stationary load feeds 4 consecutive matmuls.
                    if l > 0:
                        pp = l - 1
                        for fp_ in range(2):
                            pb = [ps.tile([128, 512], F32, tag=f"b{i}",
                                          name=f"pb{i}") for i in range(8)]
                            for g4 in range(KB // 4):
                                tch = [gp.tile([128, 512], BF16,
                                               tag=f"tc{fci}", bufs=3,
                                               name=f"tch{fci}")
                                       for fci in range(2)]
                                for fci in range(2):
                                    fc = 2 * fp_ + fci
                                    nc.sync.dma_start(
                                        tch[fci][:],
                                        agout[pp][fc]
                                        [512 * g4:512 * g4 + 512, :]
                                        .rearrange("(kk p) c -> p kk c",
                                                   p=128))
                                ar = gp.tile([128, 4 * NL], FP8, tag="ar4",
                                             bufs=2)
                                nc.scalar.dma_start(
                                    ar[:],
                                    acm[512 * g4:512 * g4 + 512, :]
                                    .rearrange("(kk p) c -> p kk c", p=128))
                                for kk in range(4):
                                    k = 4 * g4 + kk
                                    for fci in range(2):
                                        for dc in range(4):
                                            nc.tensor.matmul(
                                                pb[fci * 4 + dc][:],
                                                tch[fci][:, 128 * kk:
                                                         128 * kk + 128],
                                                ar[:, NL * kk + 512 * dc:
                                                   NL * kk + 512 * dc + 512],
                                                start=(k == 0),
                                                stop=(k == KB - 1))
                            for fci in range(2):
                                fc = 2 * fp_ + fci
                                for dc in range(4):
                                    col = 512 * dc
                                    nc.vector.tensor_tensor(
                                        out=mt[fc][:, col:col + 512],
                                        in0=pb[fci * 4 + dc][:],
                                        in1=hT[fc][:, col:col + 512],
                                        op=ALU.add)

                    # GIN MLP: u1 = relu(m@w1+b1); u2 = u1@w2
                    for j in range(4):
                        ncol = 512 * j
                        u1c = [gp.tile([128, 512], F32R, tag=f"u1_{oc}",
                                       bufs=2, name=f"u1c{oc}")
                               for oc in range(4)]
                        for oc in range(4):
                            p = ps.tile([128, 512], F32, tag=f"b{oc}")
                            for ic in range(4):
                                nc.tensor.matmul(
                                    p[:],
                                    w1s[:, 512 * ic + 128 * oc:
                                        512 * ic + 128 * oc + 128],
                                    mt[ic][:, ncol:ncol + 512],
                                    start=(ic == 0), stop=(ic == 3))
                            nc.scalar.activation(
                                u1c[oc][:], p[:], AF.Relu,
                                bias=gb1w[:, 4 * l + oc:4 * l + oc + 1])
                        for oc in range(4):
                            p = ps.tile([128, 512], F32, tag=f"b{4 + oc}")
                            for ic in range(4):
                                nc.tensor.matmul(
                                    p[:],
                                    w2s[:, 512 * ic + 128 * oc:
                                        512 * ic + 128 * oc + 128],
                                    u1c[ic][:],
                                    start=(ic == 0), stop=(ic == 3))
                            nc.vector.tensor_copy(u2T[oc][:, ncol:ncol + 512],
                                                  p[:])

                    # BN stats (local sums) -> AllReduce
                    stat = gp.tile([128, 8], F32, tag="stat")
                    for fc in range(4):
                        nc.vector.reduce_sum(stat[:, fc:fc + 1], u2T[fc][:],
                                             axis=AX.X)
                        qacc = gp.tile([128, 1], F32, tag="qacc")
                        for j in range(4):
                            sq = gp.tile([128, 512], F32, tag="sq", bufs=2)
                            nc.scalar.activation(
                                sq[:], u2T[fc][:, 512 * j:512 * j + 512],
                                AF.Square)
                            qp = gp.tile([128, 1], F32, tag=f"qp{j}")
                            nc.vector.reduce_sum(qp[:], sq[:], axis=AX.X)
                            if j == 0:
                                nc.vector.tensor_copy(qacc[:], qp[:])
                            else:
                                nc.vector.tensor_tensor(
                                    out=qacc[:], in0=qp[:], in1=qacc[:],
                                    op=ALU.add)
                        nc.vector.tensor_copy(stat[:, 4 + fc:5 + fc], qacc[:])
                    nc.sync.dma_start(bn_loc[l][:, :], stat[:])
                    nc.gpsimd.collective_compute(
                        "AllReduce", ALU.add,
                        ins=[bn_loc[l][:, :].opt()],
                        outs=[bn_glob[l][:, :].opt()],
                        replica_groups=[list(range(NCORES))],
                    )
                    # keep-warm matmuls: reading `statr` delays them to
                    # AR-trigger time so the PE stays busy (and HAM stays at
                    # full clock) through the AllReduce wait. Results unused.
                    # Skipped on the last layer where the final stage wants
                    # the PE immediately.
                    if l < L - 1:
                        statr = gp.tile([128, 8], F32R, tag="statr")
                        nc.vector.tensor_copy(statr[:], stat[:])
                        for jm in range(32):
                            pj = ps.tile([8, 512], F32, tag=f"b{jm % 2}",
                                         name="junk")
                            nc.tensor.matmul(pj[:], statr[:],
                                             w1s[:, 0:512],
                                             start=True, stop=True)
                    ga = gp.tile([128, 8], F32, tag="ga")
                    nc.sync.dma_start(ga[:], bn_glob[l][:, :])

                    # BN apply + relu + residual into hT, then chunked
                    # AllGather of the new h (bf16), one call per 128-feat
                    # chunk so the next layer's aggregation can start on
                    # chunk 0 while chunks 1-3 are still in flight.
                    for fc in range(4):
                        mu = gp.tile([128, 1], F32, tag="mu")
                        nc.vector.tensor_scalar(out=mu[:], in0=ga[:, fc:fc + 1],
                                                scalar1=1.0 / N, scalar2=None,
                                                op0=ALU.mult)
                        ex2 = gp.tile([128, 1], F32, tag="ex2")
                        nc.vector.tensor_scalar(out=ex2[:],
                                                in0=ga[:, 4 + fc:5 + fc],
                                                scalar1=1.0 / N, scalar2=None,
                                                op0=ALU.mult)
                        mu2 = gp.tile([128, 1], F32, tag="mu2")
                        nc.vector.tensor_tensor(out=mu2[:], in0=mu[:],
                                                in1=mu[:], op=ALU.mult)
                        var = gp.tile([128, 1], F32, tag="var")
                        nc.vector.tensor_tensor(out=var[:], in0=ex2[:],
                                                in1=mu2[:], op=ALU.subtract)
                        vare = gp.tile([128, 1], F32, tag="vare")
                        nc.vector.tensor_scalar(out=vare[:], in0=var[:],
                                                scalar1=BN_EPS, scalar2=None,
                                                op0=ALU.add)
                        std = gp.tile([128, 1], F32, tag="std")
                        nc.scalar.activation(std[:], vare[:], AF.Sqrt)
                        inv = gp.tile([128, 1], F32, tag="inv")
                        nc.vector.reciprocal(inv[:], std[:])
                        sv = gp.tile([128, 1], F32, tag="sv")
                        nc.vector.tensor_tensor(
                            out=sv[:], in0=inv[:],
                            in1=bngw[:, 4 * l + fc:4 * l + fc + 1],
                            op=ALU.mult)
                        mst = gp.tile([128, 1], F32, tag="mst")
                        nc.vector.tensor_tensor(out=mst[:], in0=mu[:],
                                                in1=sv[:], op=ALU.mult)
                        tv = gp.tile([128, 1], F32, tag="tv")
                        nc.vector.tensor_tensor(
                            out=tv[:], in0=bnbw[:, 4 * l + fc:4 * l + fc + 1],
                            in1=mst[:], op=ALU.subtract)
                        for j in range(4):
                            ncol = 512 * j
                            rt = gp.tile([128, 512], F32R, tag="rt", bufs=2)
                            nc.scalar.activation(
                                rt[:], u2T[fc][:, ncol:ncol + 512], AF.Relu,
                                bias=tv[:, 0:1], scale=sv[:, 0:1])
                            nc.vector.tensor_tensor(
                                out=hT[fc][:, ncol:ncol + 512], in0=rt[:],
                                in1=hT[fc][:, ncol:ncol + 512], op=ALU.add)

                        if l < L - 1:
                            pp2 = l
                            for nb4 in range(4):
                                hn = gp.tile([128, 512], BF16, tag="hn",
                                             bufs=2)
                                for nn in range(4):
                                    nb = 4 * nb4 + nn
                                    pt = ps.tile([128, 128], F32,
                                                 tag=f"b{nn % 4}")
                                    nc.tensor.transpose(
                                        pt[:],
                                        hT[fc][:, 128 * nb:128 * nb + 128]
                                        .bitcast(F32),
                                        ident[:])
                                    nc.vector.tensor_copy(
                                        hn[:, 128 * nn:128 * nn + 128], pt[:])
                                nc.sync.dma_start(
                                    agin[pp2][fc]
                                    [512 * nb4:512 * nb4 + 512, :]
                                    .rearrange("(nn p) c -> p nn c", p=128),
                                    hn[:])
                            nc.gpsimd.collective_compute(
                                "AllGather", ALU.bypass,
                                ins=[agin[pp2][fc][:, :].opt()],
                                outs=[agout[pp2][fc][:, :].opt()],
                                replica_groups=[list(range(NCORES))],
                            )

            # ---------------- final MLP + heads + pairwise ----------------
            with tc.tile_pool(name="fin", bufs=1) as fp:
                z3g = [fp.tile([ZI, 512], F32R, tag=f"z3_{g}", name=f"z3_{g}")
                       for g in range(GL)]
                hw1s = [fp.tile([ZI, ZI], F32R, tag=f"hw1_{k}", name=f"hw1s{k}")
                        for k in range(K)]
                hw2s = [fp.tile([ZI, ZI], F32R, tag=f"hw2_{k}", name=f"hw2s{k}")
                        for k in range(K)]
                for k in range(K):
                    nc.sync.dma_start(hw1s[k][:], hw1[ZI * k:ZI * k + ZI, :])
                    nc.sync.dma_start(hw2s[k][:], hw2[ZI * k:ZI * k + ZI, :])

                # phase A: node MLP down to z3, graph-pairs so each
                # stationary weight load feeds 2 matmuls (graphs innermost).
                with tc.tile_pool(name="finA", bufs=1) as fa:
                    mwa_all = fa.tile([128, 4 * DFF], F32R, tag="mwa")
                    nc.sync.dma_start(
                        mwa_all[:],
                        mw1[:, :].rearrange("(ic p) c -> p ic c", p=128))
                    mwb_all = fa.tile([128, 8 * DFF], F32R, tag="mwb")
                    nc.scalar.dma_start(
                        mwb_all[:],
                        mw2[:, :].rearrange("(ic p) c -> p ic c", p=128))
                    mwc_all = fa.tile([128, 8 * ZI], F32R, tag="mwc")
                    nc.sync.dma_start(
                        mwc_all[:],
                        mw3[:, :].rearrange("(ic p) c -> p ic c", p=128))
                    mwa = [mwa_all[:, DFF * ic:DFF * ic + DFF]
                           for ic in range(4)]
                    mwb = [mwb_all[:, DFF * ic:DFF * ic + DFF]
                           for ic in range(8)]
                    mwc = [mwc_all[:, ZI * ic:ZI * ic + ZI]
                           for ic in range(8)]

                    for gp2 in range(GL // 2):
                        gs = (2 * gp2, 2 * gp2 + 1)
                        z1 = [[fa.tile([128, 512], F32R, tag=f"z1_{gj}_{oc}",
                                       name=f"z1_{gj}_{oc}")
                               for oc in range(8)] for gj in range(2)]
                        for oh in range(2):
                            pb2 = [[ps.tile([128, 512], F32,
                                            tag=f"b{2 * oi + gj}",
                                            name=f"pb2_{oi}_{gj}")
                                    for gj in range(2)] for oi in range(4)]
                            for ic in range(4):
                                for oi in range(4):
                                    oc = 4 * oh + oi
                                    for gj in range(2):
                                        nc.tensor.matmul(
                                            pb2[oi][gj][:],
                                            mwa_all[:, DFF * ic + 128 * oc:DFF * ic + 128 * oc + 128],
                                            hT[ic][:, 512 * gs[gj]:
                                                    512 * gs[gj] + 512],
                                            start=(ic == 0), stop=False)
                            for oi in range(4):
                                oc = 4 * oh + oi
                                for gj in range(2):
                                    nc.tensor.matmul(
                                        pb2[oi][gj][:],
                                        mb1rw[:, 128 * oc:128 * oc + 128],
                                        onf[:, 0:512], start=False, stop=True,
                                        skip_group_check=True)
                                    nc.vector.tensor_scalar(
                                        out=z1[gj][oc][:], in0=pb2[oi][gj][:],
                                        scalar1=0.0, scalar2=None, op0=ALU.max)
                        z2 = [[fa.tile([128, 512], F32R, tag=f"z2_{gj}_{oc}",
                                       name=f"z2_{gj}_{oc}")
                               for oc in range(8)] for gj in range(2)]
                        for oh in range(2):
                            pb2 = [[ps.tile([128, 512], F32,
                                            tag=f"b{2 * oi + gj}",
                                            name=f"pc2_{oi}_{gj}")
                                    for gj in range(2)] for oi in range(4)]
                            for ic in range(8):
                                for oi in range(4):
                                    oc = 4 * oh + oi
                                    for gj in range(2):
                                        nc.tensor.matmul(
                                            pb2[oi][gj][:],
                                            mwb_all[:, DFF * ic + 128 * oc:DFF * ic + 128 * oc + 128],
                                            z1[gj][ic][:],
                                            start=(ic == 0), stop=False)
                            for oi in range(4):
                                oc = 4 * oh + oi
                                for gj in range(2):
                                    nc.tensor.matmul(
                                        pb2[oi][gj][:],
                                        mb2rw[:, 128 * oc:128 * oc + 128],
                                        onf[:, 0:512], start=False, stop=True,
                                        skip_group_check=True)
                                    nc.vector.tensor_scalar(
                                        out=z2[gj][oc][:], in0=pb2[oi][gj][:],
                                        scalar1=0.0, scalar2=None, op0=ALU.max)
                        pz2 = [ps.tile([ZI, 512], F32, tag=f"b{gj}",
                                       name=f"pz2_{gj}")
                               for gj in range(2)]
                        for ic in range(8):
                            for gj in range(2):
                                nc.tensor.matmul(pz2[gj][:],
                                                 mwc[ic],
                                                 z2[gj][ic][:],
                                                 start=(ic == 0),
                                                 stop=(ic == 7))
                        for gj in range(2):
                            nc.vector.tensor_tensor(
                                out=z3g[gs[gj]][:], in0=pz2[gj][:],
                                in1=mb3w[:, 0:1].to_broadcast([ZI, 512])[:],
                                op=ALU.add)

                # phase B/C over two k-halves: heads batched so each
                # stationary load feeds all 4 graphs, then pairwise with
                # batched Ln/Sigmoid (few activation-table swaps).
                with tc.tile_pool(name="finB", bufs=1) as fb:
                    for hf in range(2):
                        idx = [(g, k) for k in (2 * hf, 2 * hf + 1)
                               for g in range(GL)]
                        hkh8 = [fb.tile([ZI, 512], BF16, tag=f"hh{j}",
                                        name=f"hh{j}") for j in range(8)]
                        hkl8 = [fb.tile([ZI, 512], BF16, tag=f"hl{j}",
                                        name=f"hl{j}") for j in range(8)]
                        rsh8 = [fb.tile([1, 512], BF16, tag=f"rh{j}",
                                        name=f"rh{j}") for j in range(8)]
                        rsl8 = [fb.tile([1, 512], BF16, tag=f"rl{j}",
                                        name=f"rl{j}") for j in range(8)]
                        h18 = [fb.tile([ZI, 512], F32R, tag=f"h1_{j}",
                                       name=f"h1_{j}") for j in range(8)]
                        for kj in range(2):
                            k = 2 * hf + kj
                            p14 = [ps.tile([ZI, 512], F32,
                                           tag=f"b{4 * kj + g}",
                                           name=f"p14_{g}")
                                   for g in range(GL)]
                            for g in range(GL):
                                nc.tensor.matmul(p14[g][:], hw1s[k][:],
                                                 z3g[g][:],
                                                 start=True, stop=False)
                            for g in range(GL):
                                nc.tensor.matmul(
                                    p14[g][:], hb1rw[:, ZI * k:ZI * k + ZI],
                                    onf[:, 0:512], start=False, stop=True,
                                    skip_group_check=True)
                                nc.vector.tensor_scalar(
                                    out=h18[4 * kj + g][:], in0=p14[g][:],
                                    scalar1=0.0, scalar2=None, op0=ALU.max)
                        for kj in range(2):
                            k = 2 * hf + kj
                            p24 = [ps.tile([ZI, 512], F32,
                                           tag=f"b{4 * kj + g}",
                                           name=f"p24_{g}")
                                   for g in range(GL)]
                            for g in range(GL):
                                j = 4 * kj + g
                                nc.tensor.matmul(p24[g][:], hw2s[k][:],
                                                 h18[j][:],
                                                 start=True, stop=True)
                            for g in range(GL):
                                j = 4 * kj + g
                                hkt = fb.tile([ZI, 512], F32, tag="hkt",
                                              bufs=2)
                                nc.vector.tensor_tensor(
                                    out=hkt[:], in0=p24[g][:],
                                    in1=hb2w[:, k:k + 1]
                                    .to_broadcast([ZI, 512])[:],
                                    op=ALU.add)
                                nc.vector.tensor_copy(hkh8[j][:], hkt[:])
                                nc.vector.tensor_tensor(out=hkl8[j][:],
                                                        in0=hkt[:],
                                                        in1=hkh8[j][:],
                                                        op=ALU.subtract)
                                sqt = fb.tile([ZI, 512], F32, tag="sqt",
                                              bufs=2)
                                nc.vector.tensor_tensor(out=sqt[:],
                                                        in0=hkt[:],
                                                        in1=hkt[:],
                                                        op=ALU.mult)
                                pr = ps.tile([1, 512], F32, tag=f"b{j % 2}",
                                             name="pr")
                                nc.tensor.matmul(pr[:], ofn[:], sqt[:],
                                                 start=True, stop=True)
                                rsb = fb.tile([1, 512], F32, tag="rsb",
                                              bufs=2)
                                nc.vector.tensor_copy(rsb[:], pr[:])
                                nc.vector.tensor_copy(rsh8[j][:], rsb[:])
                                nc.vector.tensor_tensor(out=rsl8[j][:],
                                                        in0=rsb[:],
                                                        in1=rsh8[j][:],
                                                        op=ALU.subtract)
                        # pairwise: pd = h_i.h_j - r_i/2 - r_j/2 = -d2/2
                        for q in range(4):
                            d2p = [fb.tile([128, 2048], F32, tag=f"d2_{j}",
                                           name=f"d2_{j}") for j in range(2)]
                            for j in range(2):
                                w = 2 * q + j
                                hh_ = hkh8[w]
                                hl_ = hkl8[w]
                                rh_ = rsh8[w]
                                rl_ = rsl8[w]
                                for mb in range(4):
                                    c0 = 128 * mb
                                    pd = ps.tile([128, 512], F32,
                                                 tag=f"b{4 + (j * 4 + mb) % 2}")
                                    nc.tensor.matmul(
                                        pd[:], hh_[:, c0:c0 + 128], hh_[:],
                                        start=True, stop=False)
                                    nc.tensor.matmul(
                                        pd[:], hh_[:, c0:c0 + 128], hl_[:],
                                        start=False, stop=False,
                                        skip_group_check=True)
                                    nc.tensor.matmul(
                                        pd[:], hl_[:, c0:c0 + 128], hh_[:],
                                        start=False, stop=False,
                                        skip_group_check=True)
                                    nc.tensor.matmul(
                                        pd[:], onfb[:, 0:128], rh_[:],
                                        start=False, stop=False,
                                        skip_group_check=True)
                                    nc.tensor.matmul(
                                        pd[:], onfb[:, 0:128], rl_[:],
                                        start=False, stop=False,
                                        skip_group_check=True)
                                    nc.tensor.matmul(
                                        pd[:], rh_[:, c0:c0 + 128], onfb[:],
                                        start=False, stop=False,
                                        skip_group_check=True)
                                    nc.tensor.matmul(
                                        pd[:], rl_[:, c0:c0 + 128], onfb[:],
                                        start=False, stop=True,
                                        skip_group_check=True)
                                    nc.vector.tensor_scalar(
                                        out=d2p[j][:, 512 * mb:512 * mb + 512],
                                        in0=pd[:], scalar1=-2.0,
                                        scalar2=1e-12, op0=ALU.mult,
                                        op1=ALU.max)
                            for jm in range(6):
                                pj2 = ps.tile([ZI, 512], F32,
                                              tag=f"b{jm % 2}", name="pj2")
                                nc.tensor.matmul(pj2[:], hw1s[0][:],
                                                 z3g[0][:],
                                                 start=True, stop=True)
                            lnp = [fb.tile([128, 2048], F32, tag=f"ln_{j}",
                                           name=f"ln_{j}") for j in range(2)]
                            for j in range(2):
                                nc.scalar.activation(lnp[j][:], d2p[j][:],
                                                     AF.Ln)
                            for j in range(2):
                                w = 2 * q + j
                                g, k = idx[w]
                                qt = fb.tile([128, 2048], F32, tag="qt",
                                             bufs=2)
                                nc.scalar.activation(qt[:], lnp[j][:],
                                                     AF.Sigmoid,
                                                     bias=nla[:, 0:1],
                                                     scale=-UMAP_B)
                                row = (g * K + k) * 4 * 128
                                nc.sync.dma_start(
                                    qout[row:row + 512, :]
                                    .rearrange("(mb p) c -> p mb c", p=128),
                                    qt[:])
    nc.compile()
    return nc


def _host_prep(inputs):
    x = np.asarray(inputs["x"], np.float32)
    edge_index = np.asarray(inputs["edge_index"], np.int64)
    src, dst = edge_index[0], edge_index[1]

    x_aug = np.concatenate([x, np.ones((N, 1), np.float32)], axis=1)  # [N,10]
    xn_host = np.ascontiguousarray(
        x_aug.reshape(KB, 128, 10).transpose(1, 0, 2).reshape(128, KB * 10)
    ).astype(ml_dtypes.bfloat16)

    shared = {
        "xn": xn_host,
        "embw": np.ascontiguousarray(np.vstack(
            [np.asarray(inputs["emb_w"], np.float32),
             np.asarray(inputs["emb_b"], np.float32)[None, :]])),
        "gw1": np.ascontiguousarray(
            np.asarray(inputs["gin_w1"], np.float32).reshape(L * D, D)),
        "gw2": np.ascontiguousarray(
            np.asarray(inputs["gin_w2"], np.float32).reshape(L * D, D)),
        "mw1": np.ascontiguousarray(np.asarray(inputs["mlp_w1"], np.float32)),
        "mw2": np.ascontiguousarray(np.asarray(inputs["mlp_w2"], np.float32)),
        "mw3": np.ascontiguousarray(np.asarray(inputs["mlp_w3"], np.float32)),
        "hw1": np.ascontiguousarray(
            np.asarray(inputs["head_w1"], np.float32).reshape(K * ZI, ZI)),
        "hw2": np.ascontiguousarray(
            np.asarray(inputs["head_w2"], np.float32).reshape(K * ZI, ZI)),
        "gb1_d": np.ascontiguousarray(
            np.asarray(inputs["gin_b1"], np.float32)
            .reshape(L, 4, 128).transpose(2, 0, 1).reshape(128, 16)),
        "bng_d": np.ascontiguousarray(
            np.asarray(inputs["bn_g"], np.float32)
            .reshape(L, 4, 128).transpose(2, 0, 1).reshape(128, 16)),
        "bnb_d": np.ascontiguousarray(
            np.asarray(inputs["bn_b"], np.float32)
            .reshape(L, 4, 128).transpose(2, 0, 1).reshape(128, 16)),
        "mb1_d": np.ascontiguousarray(
            np.asarray(inputs["mlp_b1"], np.float32).reshape(8, 128).T),
        "mb2_d": np.ascontiguousarray(
            np.asarray(inputs["mlp_b2"], np.float32).reshape(8, 128).T),
        "mb3_d": np.ascontiguousarray(
            np.asarray(inputs["mlp_b3"], np.float32)[:, None]),
        "hb1_d": np.ascontiguousarray(
            np.asarray(inputs["head_b1"], np.float32).T),
        "hb2_d": np.ascontiguousarray(
            np.asarray(inputs["head_b2"], np.float32).T),
        "mb1_r": np.ascontiguousarray(
            np.asarray(inputs["mlp_b1"], np.float32)[None, :]),
        "mb2_r": np.ascontiguousarray(
            np.asarray(inputs["mlp_b2"], np.float32)[None, :]),
        "hb1_r": np.ascontiguousarray(
            np.asarray(inputs["head_b1"], np.float32).reshape(1, K * ZI)),
    }

    in_maps = []
    ones_row = np.ones((1, NL), np.float32)
    for c in range(NCORES):
        lo = NL * c
        mask = (dst >= lo) & (dst < lo + NL)
        flat = src[mask] * NL + (dst[mask] - lo)
        a = np.bincount(flat, minlength=N * NL).astype(np.float32)
        m = dict(shared)
        m["acm"] = np.ascontiguousarray(a.reshape(N, NL)).astype(
            ml_dtypes.float8_e4m3)
        m["xt"] = np.ascontiguousarray(
            np.vstack([x[lo:lo + NL].T, ones_row]))
        in_maps.append(m)
    return in_maps


def kernel(**inputs) -> np.ndarray:
    global _NC_CACHE
    if _NC_CACHE is None:
        _NC_CACHE = build_nc()
    nc = _NC_CACHE
    in_maps = _host_prep(inputs)
    res = run_bass_kernel_spmd(nc, in_maps, core_ids=list(range(NCORES)))
    out = np.concatenate(
        [np.asarray(res.results[c]["qout"]).reshape(GL, K, NG, NG)
         for c in range(NCORES)], axis=0)
    return out
